# revision 1
# baseline (speedup 1.0000x reference)
"""FFM layer (nn_FFM_Layer) Trainium2 Bass kernel.

Reference computation (B=4096, 13 dense fields, 26 sparse fields with vocab
1000 each, FIELD_NUM=39, K=16):

    idx        = sparse + offsets                      # [B, 26] global ids
    first      = w0 + dense @ w[:13] + sum_j w[idx]    # [B, 1]
    field_f    = einsum('bd,dfk', dense, v[:13]) + sum_j v[idx]   # [B,39,16]
    s          = field_f.sum(1)                        # [B, 16]
    second     = 0.5*(||s||^2 - sum_fk field_f^2)      # [B]
    out        = first + second[:, None]

Strategy (data-parallel over batch, 8 cores x 512 samples, no collectives):
  * Host packs an augmented table V_AUG [26013, 640] f32:
      cols [0:624]  = v.reshape(26013, 39*16)
      col  624      = w[:, 0]   (+ w0 folded into rows of sparse table 0,
                                 which every sample hits exactly once)
      cols [625:640]= 0         (pad rows to 2560 B; dma_gather requires
                                 elem_size_bytes % 256 == 0)
  * Each core dma_gathers its 512*26 rows (SWDGE mlp ucode): one gathered
    row brings both the v-row and its w contribution.  Gathers are
    sample-chunk-major (4-5 calls of 2-7 fields x 128 samples per chunk)
    so each chunk's FM epilogue + output DMA overlap later chunks.
    HW-measured constraints baked in here:
      - one call tops out at ~1024 descriptors (1536+ wedges the exec unit)
      - Q7 desc-gen runs ~8-9 ns/row; two SWDGE queues (num_swdge_queues=2,
        alternating queue_num) overlap desc-gen and speed SDMA draining
      - single_packet=False is ~10% faster end to end
      - first/last chunk use smaller first/last calls to shorten pipeline
        fill and drain
  * DVE folds each call's 2-7 gathered cols into one col (pairwise adds;
    strided tensor_reduce over 2560B-stride views measured 3x slower).
  * PE seeds each chunk's PSUM with the dense [13,128]x[13,640] matmul
    (dense^T prepared host-side; col 624 adds dense @ w[:13]) and
    accumulates the fold cols via identity matmuls; the final col is
    added by DVE together with the single PSUM read.
  * FM identity epilogue per chunk: ACT Square+accum_out for both norms
    (InstTensorTensorReduce wedges the exec unit on this HW path), one
    strided DVE tensor_reduce over [128,16,39] for the 39-field s-sum.

Measured: ~127-135 us on HW (min 126.3) vs ~100 us SWDGE-DMA active time
at ~336 GB/s effective; the gather stream is simultaneously near the
per-core DMA-bus, chip-HBM, and Q7 descriptor-generation walls.
"""

import sys

if "/opt/trn_rl_repo" not in sys.path:
    sys.path.insert(0, "/opt/trn_rl_repo")

import numpy as np

import concourse.bacc as bacc
import concourse.bass as bass
import concourse.tile as tile
from concourse import mybir
from concourse.bass_utils import run_bass_kernel_spmd

# Problem constants (hardcoded per harness contract)
B = 4096
N_DENSE = 13
N_SPARSE = 26
FEAT_PER_SPARSE = 1000
FIELD_NUM = 39
FEATURE_NUM = 26013
K = 16
N_CORES = 8
BC = B // N_CORES          # 512 samples per core
ROW = 640                  # padded row: 624 v + 1 w + 15 zeros (2560 B)
VCOLS = FIELD_NUM * K      # 624
P = 128
SCHUNKS = BC // P          # 4 sample chunks of 128 per core
# per-chunk gather calls: field groups (sum 26), each call = nf*128 idxs
FGROUPS = [7, 7, 6, 6]
FGROUPS_FIRST = [2, 6, 6, 6, 6]
FGROUPS_LAST = [7, 7, 6, 4, 2]
IDX_COLS_SC = N_SPARSE * P // 16   # 208 idx cols per sample chunk

F32 = mybir.dt.float32
I16 = mybir.dt.int16


def build_program():
    """Build + compile the single-core SPMD bass program."""
    nc = bacc.Bacc("TRN2", target_bir_lowering=False, debug=False,
                   num_swdge_queues=2)

    vaug_t = nc.dram_tensor("vaug", [FEATURE_NUM, ROW], F32, kind="ExternalInput")
    dense_t = nc.dram_tensor("dense_t", [N_DENSE, BC], F32, kind="ExternalInput")
    idxs_t = nc.dram_tensor("idxs", [P, SCHUNKS * IDX_COLS_SC], I16,
                            kind="ExternalInput")
    ident_t = nc.dram_tensor("ident", [P, P], F32, kind="ExternalInput")
    out_t = nc.dram_tensor("out", [P, SCHUNKS], F32, kind="ExternalOutput")

    with tile.TileContext(nc) as tc:
        with (
            tc.tile_pool(name="main", bufs=1) as main,
            tc.tile_pool(name="gath", bufs=7) as gath,
            tc.tile_pool(name="fold", bufs=3) as fold,
            tc.tile_pool(name="small", bufs=2) as small,
            tc.tile_pool(name="psum", bufs=4, space="PSUM") as psum,
        ):
            # per-sample-chunk idx tiles so the first gather starts early
            idx_sbs = []
            for c in range(SCHUNKS):
                t = main.tile([P, IDX_COLS_SC], I16, tag=f"idx{c}")
                nc.sync.dma_start(
                    t[:], idxs_t[:, c * IDX_COLS_SC : (c + 1) * IDX_COLS_SC]
                )
                idx_sbs.append(t)
            vaug13 = main.tile([N_DENSE, ROW], F32)
            nc.sync.dma_start(vaug13[:], vaug_t[0:N_DENSE, :])
            dt_sb = main.tile([N_DENSE, BC], F32)
            nc.sync.dma_start(dt_sb[:], dense_t[:])
            ident = main.tile([P, P], F32)
            nc.sync.dma_start(ident[:], ident_t[:])

            res = main.tile([P, SCHUNKS], F32)

            call_no = 0
            for c in range(SCHUNKS):
                # dense part seeds this chunk's PSUM accumulation chain;
                # PE also accumulates the first two fold cols into it.
                ps = psum.tile([P, ROW], F32, tag="ps")
                lhs_d = dt_sb[:, c * P : (c + 1) * P]
                nc.tensor.matmul(out=ps[:, 0:512], lhsT=lhs_d,
                                 rhs=vaug13[:, 0:512], start=True, stop=False)
                nc.tensor.matmul(out=ps[:, 512:ROW], lhsT=lhs_d,
                                 rhs=vaug13[:, 512:ROW], start=True, stop=False)

                if c == 0:
                    fgroups = FGROUPS_FIRST
                elif c == SCHUNKS - 1:
                    fgroups = FGROUPS_LAST
                else:
                    fgroups = FGROUPS
                t2s = []
                icol = 0
                for gi, nf in enumerate(fgroups):
                    n_idx = nf * P
                    g = gath.tile([P, 7, ROW], F32, tag="g")
                    nc.gpsimd.dma_gather(
                        g[:, :nf, :],
                        vaug_t[:],
                        idx_sbs[c][:, icol : icol + n_idx // 16],
                        n_idx,
                        n_idx,
                        ROW,
                        single_packet=False,
                        queue_num=call_no % 2,
                    )
                    icol += n_idx // 16
                    call_no += 1
                    # DVE fold: call's nf cols -> t2
                    t1 = fold.tile([P, 3, ROW], F32, tag="t1")
                    t2 = small.tile([P, ROW], F32, tag=f"t2_{gi}")
                    if nf >= 6:
                        nc.vector.tensor_add(t1[:], g[:, 0:3, :], g[:, 3:6, :])
                        nc.vector.tensor_add(t2[:], t1[:, 0, :], t1[:, 1, :])
                        nc.vector.tensor_add(t2[:], t2[:], t1[:, 2, :])
                        if nf == 7:
                            nc.vector.tensor_add(t2[:], t2[:], g[:, 6, :])
                    elif nf == 4:
                        nc.vector.tensor_add(t1[:, 0:2, :], g[:, 0:2, :], g[:, 2:4, :])
                        nc.vector.tensor_add(t2[:], t1[:, 0, :], t1[:, 1, :])
                    else:  # nf == 2
                        nc.vector.tensor_add(t2[:], g[:, 0, :], g[:, 1, :])
                    t2s.append(t2)
                    # PE accumulates fold cols into the psum chain; the
                    # final col goes via DVE to keep PE out of the tail
                    if gi < len(fgroups) - 1:
                        last = gi == len(fgroups) - 2
                        nc.tensor.matmul(out=ps[:, 0:512], lhsT=ident[:],
                                         rhs=t2[:, 0:512],
                                         start=False, stop=last)
                        nc.tensor.matmul(out=ps[:, 512:ROW], lhsT=ident[:],
                                         rhs=t2[:, 512:ROW],
                                         start=False, stop=last)

                # fld = psum chain + last col (single PSUM read on DVE)
                fld = fold.tile([P, ROW], F32, tag="fld")
                nc.vector.tensor_add(fld[:], t2s[-1][:], ps[:])

                # --- FM identity epilogue for this chunk ---
                blk = fld[:, 0:VCOLS]             # [128, 624] = field_f
                sq = fold.tile([P, VCOLS], F32, tag="sq")
                q = small.tile([P, 1], F32, tag="q")
                nc.scalar.activation(
                    sq[:], blk, mybir.ActivationFunctionType.Square,
                    accum_out=q[:],
                )
                # s = sum over the 39 fields: strided reduce of [128,16,39]
                st = fold.tile([P, 16], F32, tag="st")
                blk_kf = blk.rearrange("p (f k) -> p k f", k=16)
                nc.vector.tensor_reduce(
                    out=st[:], in_=blk_kf, op=mybir.AluOpType.add,
                    axis=mybir.AxisListType.X,
                )
                s2 = small.tile([P, 16], F32, tag="s2")
                snorm = small.tile([P, 1], F32, tag="snorm")
                nc.scalar.activation(
                    s2[:], st[:], mybir.ActivationFunctionType.Square,
                    accum_out=snorm[:],
                )
                diff = small.tile([P, 1], F32, tag="diff")
                nc.vector.tensor_tensor(
                    out=diff[:], in0=snorm[:], in1=q[:],
                    op=mybir.AluOpType.subtract,
                )
                # out = 0.5*diff + (w-sum incl. w0 and dense first-order)
                nc.scalar.activation(
                    res[:, c : c + 1],
                    diff[:],
                    mybir.ActivationFunctionType.Identity,
                    bias=fld[:, VCOLS : VCOLS + 1],
                    scale=0.5,
                )
                nc.sync.dma_start(out_t[:, c : c + 1], res[:, c : c + 1])

    nc.compile()
    return nc


def prep_inputs(dense_inputs, sparse_inputs, w0, w, v):
    """Host-side shard/pack: build per-core in_maps."""
    dense = np.asarray(dense_inputs, np.float32)
    sparse = np.asarray(sparse_inputs)
    w0 = np.asarray(w0, np.float32)
    w = np.asarray(w, np.float32)
    v = np.asarray(v, np.float32)

    vaug = np.zeros((FEATURE_NUM, ROW), np.float32)
    vaug[:, :VCOLS] = v.reshape(FEATURE_NUM, VCOLS)
    vaug[:, VCOLS] = w[:, 0]
    # fold w0 into sparse table 0 (each sample hits it exactly once)
    vaug[N_DENSE : N_DENSE + FEAT_PER_SPARSE, VCOLS] += w0[0]

    offs = N_DENSE + FEAT_PER_SPARSE * np.arange(N_SPARSE, dtype=np.int64)
    gidx = (sparse.astype(np.int64) + offs[None, :]).astype(np.int16)  # [B, 26]

    in_maps = []
    for core in range(N_CORES):
        sl = slice(core * BC, (core + 1) * BC)
        dt = np.ascontiguousarray(dense[sl].T)          # [13, 512]
        idxc = gidx[sl]                                 # [512, 26]
        buf = np.zeros((P, SCHUNKS * IDX_COLS_SC), np.int16)
        off_c = 0
        for c in range(SCHUNKS):
            rows = idxc[c * P : (c + 1) * P]            # [128, 26]
            fbase = 0
            for nf in (FGROUPS_FIRST if c == 0 else
                       (FGROUPS_LAST if c == SCHUNKS - 1 else FGROUPS)):
                n = nf * P
                # call order: i = f_local*128 + p  ->  row idx[p, fbase+f]
                seg = np.ascontiguousarray(
                    rows[:, fbase : fbase + nf].T
                ).reshape(-1)                           # [nf*128]
                wrapped = seg.reshape(n // 16, 16).T    # [16, n/16]
                buf[:, off_c : off_c + n // 16] = np.tile(wrapped, (8, 1))
                fbase += nf
                off_c += n // 16
        in_maps.append({"vaug": vaug, "dense_t": dt, "idxs": buf,
                        "ident": np.eye(P, dtype=np.float32)})
    return in_maps


_NC_CACHE = None


def kernel(dense_inputs, sparse_inputs, w0, w, v):
    global _NC_CACHE
    if _NC_CACHE is None:
        _NC_CACHE = build_program()
    nc = _NC_CACHE
    in_maps = prep_inputs(dense_inputs, sparse_inputs, w0, w, v)
    res = run_bass_kernel_spmd(nc, in_maps, core_ids=list(range(N_CORES)))
    outs = []
    for r in res.results:
        o = r["out"]                                    # [128, 4]
        outs.append(np.ascontiguousarray(o.T).reshape(BC, 1))
    return np.concatenate(outs, axis=0).astype(np.float32)



# revision 25
# speedup vs baseline: 1.5216x; 1.5216x over previous
"""FFM layer (nn_FFM_Layer) Trainium2 Bass kernel.

Reference computation (B=4096, 13 dense fields, 26 sparse fields with vocab
1000 each, FIELD_NUM=39, K=16):

    idx        = sparse + offsets                      # [B, 26] global ids
    first      = w0 + dense @ w[:13] + sum_j w[idx]    # [B, 1]
    field_f    = einsum('bd,dfk', dense, v[:13]) + sum_j v[idx]   # [B,39,16]
    s          = field_f.sum(1)                        # [B, 16]
    second     = 0.5*(||s||^2 - sum_fk field_f^2)      # [B]
    out        = first + second[:, None]

Strategy (data-parallel over batch, 8 cores x 512 samples, no collectives):
  * Host packs an augmented table V_AUG [26013, 640] in FP16 (1280 B rows;
    dma_gather needs elem bytes % 256 == 0):
      cols [0:624]  = v.reshape(26013, 39*16)
      col  624      = fp16 hi half of w[:, 0] (+ w0 folded into sparse
                      table 0, which every sample hits exactly once)
      col  625      = fp16 lo residual of w (w - fp32(hi)), so the
                      first-order sum is ~fp32-exact
      cols [626:640]= 0
    FP16 halves gather traffic vs fp32 (the baseline's wall); its 11-bit
    mantissa keeps the second-order output error ~5e-4 rms relative.
    (FP8-E3M4 rows were tried: 768 B rows drain at only ~232 GB/s vs
    ~316 GB/s for 1280 B rows, and the fp8->fp16 level-1 DVE adds lose
    the 2x 16-bit mode; net SLOWER. fp16 is the sweet spot.)
  * Each core dma_gathers its 512*26 rows (SWDGE mlp ucode), 4 SWDGE
    queues round-robin: queue q's desc-gen runs on GPSIMD cpu pair
    2q/2q+1, so 4 queues generate concurrently (~6ns/row each), paced by
    SDMA ring backpressure. HW-measured: one call tops out at ~1024
    descriptors; single_packet=True is ~5% faster at 1280 B rows (the
    opposite held at 2560 B); gath pool bufs=11 keeps the rings fed
    ~4 calls ahead of the DVE folds.
  * Head: chunk-0's idx block (53 KB) loads first and alone, then the
    chunk-0 gathers issue, then the remaining idx + aux tensors load.
  * DVE folds each call's 2-7 gathered cols into one col (pairwise adds,
    fp16 in/out for the 2x_1P mode); PE accumulates the fold cols into
    fp32 PSUM via fp16 identity matmuls on top of the dense seed; the
    final col is added by DVE together with the single PSUM read.
  * Dense seed: one K=39 fp16 matmul with Dekker hi/lo splitting
    (rows [x_hi; x_lo; x_hi] against [A_hi; A_hi; A_lo]) gives x@A to
    ~2^-22 relative: A = [v_dense | w13 | 0].
  * FM identity epilogue per chunk: ACT Square+accum_out for both norms,
    one strided DVE tensor_reduce over [128,16,39] for the 39-field
    s-sum.
"""

import sys

if "/opt/trn_rl_repo" not in sys.path:
    sys.path.insert(0, "/opt/trn_rl_repo")

import numpy as np

import concourse.bacc as bacc
import concourse.bass as bass
import concourse.tile as tile
from concourse import mybir
from concourse.bass_utils import run_bass_kernel_spmd

# Problem constants (hardcoded per harness contract)
B = 4096
N_DENSE = 13
N_SPARSE = 26
FEAT_PER_SPARSE = 1000
FIELD_NUM = 39
FEATURE_NUM = 26013
K = 16
N_CORES = 8
BC = B // N_CORES          # 512 samples per core
ROW = 640                  # padded row: 624 v + w_hi + w_lo + 14 zeros (1280 B)
VCOLS = FIELD_NUM * K      # 624
P = 128
SCHUNKS = BC // P          # 4 sample chunks of 128 per core
# per-chunk gather calls: field groups (sum 26), each call = nf*128 idxs.
# Groups never straddle the 13-col half boundary: calls land in two
# [128, 13, 640] half tiles so the fold is 4 wide DVE adds per half.
FGROUPS = [7, 6, 7, 6]
FGROUPS_FIRST = [2, 5, 6, 7, 6]
FGROUPS_LAST = [7, 6, 7, 4, 2]
IDX_COLS_SC = N_SPARSE * P // 16   # 208 idx cols per sample chunk
N_QUEUES = 4

F32 = mybir.dt.float32
F16 = mybir.dt.float16
I16 = mybir.dt.int16


def groups_of(c):
    if c == 0:
        return FGROUPS_FIRST
    if c == SCHUNKS - 1:
        return FGROUPS_LAST
    return FGROUPS


def build_program():
    """Build + compile the single-core SPMD bass program."""
    nc = bacc.Bacc("TRN2", target_bir_lowering=False, debug=False,
                   num_swdge_queues=N_QUEUES)

    vaug_t = nc.dram_tensor("vaug", [FEATURE_NUM, ROW], F16, kind="ExternalInput")
    dense_t = nc.dram_tensor("dense_t", [3 * N_DENSE, BC], F16,
                             kind="ExternalInput")
    vdense_t = nc.dram_tensor("vdense", [3 * N_DENSE, ROW], F16,
                              kind="ExternalInput")
    idxs_t = nc.dram_tensor("idxs", [P, SCHUNKS * IDX_COLS_SC], I16,
                            kind="ExternalInput")
    ident_t = nc.dram_tensor("ident", [P, P], F16, kind="ExternalInput")
    out_t = nc.dram_tensor("out", [P, SCHUNKS], F32, kind="ExternalOutput")

    with tile.TileContext(nc) as tc:
        with (
            tc.tile_pool(name="main", bufs=1) as main,
            tc.tile_pool(name="gath", bufs=8) as gath,
            tc.tile_pool(name="fold", bufs=2) as fold,
            tc.tile_pool(name="small", bufs=2) as small,
            tc.tile_pool(name="psum", bufs=4, space="PSUM") as psum,
        ):
            # chunk-0 idx block loads first (small, gates the first gather)
            idx_sb = main.tile([P, SCHUNKS * IDX_COLS_SC], I16)
            nc.sync.dma_start(idx_sb[:, 0:IDX_COLS_SC], idxs_t[:, 0:IDX_COLS_SC])

            res = main.tile([P, SCHUNKS], F32)

            call_no = 0
            gtiles = {}

            def issue_gathers(c):
                nonlocal call_no
                icol = c * IDX_COLS_SC
                hA = gath.tile([P, 13, ROW], F16, tag="h")
                hB = gath.tile([P, 13, ROW], F16, tag="h")
                fbase = 0
                for nf in groups_of(c):
                    n_idx = nf * P
                    if fbase < 13:
                        tgt, off = hA, fbase
                    else:
                        tgt, off = hB, fbase - 13
                    nc.gpsimd.dma_gather(
                        tgt[:, off : off + nf, :],
                        vaug_t[:],
                        idx_sb[:, icol : icol + n_idx // 16],
                        n_idx,
                        n_idx,
                        ROW,
                        single_packet=True,
                        queue_num=call_no % N_QUEUES,
                    )
                    icol += n_idx // 16
                    fbase += nf
                    call_no += 1
                gtiles[c] = (hA, hB)

            issue_gathers(0)

            # remaining idx blocks + aux loads (not needed by chunk-0 gathers)
            nc.sync.dma_start(idx_sb[:, IDX_COLS_SC:],
                              idxs_t[:, IDX_COLS_SC:])
            vdense = main.tile([3 * N_DENSE, ROW], F16)
            nc.sync.dma_start(vdense[:], vdense_t[:])
            dt_sb = main.tile([3 * N_DENSE, BC], F16)
            nc.sync.dma_start(dt_sb[:], dense_t[:])
            ident = main.tile([P, P], F16)
            nc.sync.dma_start(ident[:], ident_t[:])

            for c in range(SCHUNKS):
                if c not in gtiles:
                    issue_gathers(c)
                for ahead in (c + 1, c + 2):
                    if ahead < SCHUNKS and ahead not in gtiles:
                        issue_gathers(ahead)
                fgroups = groups_of(c)

                # dense part seeds this chunk's PSUM accumulation chain
                ps = psum.tile([P, ROW], F32, tag="ps")
                lhs_d = dt_sb[:, c * P : (c + 1) * P]
                nc.tensor.matmul(out=ps[:, 0:512], lhsT=lhs_d,
                                 rhs=vdense[:, 0:512], start=True, stop=False)
                nc.tensor.matmul(out=ps[:, 512:ROW], lhsT=lhs_d,
                                 rhs=vdense[:, 512:ROW], start=True, stop=False)

                hA, hB = gtiles[c]

                # wide DVE fold: 13 cols -> 1 in four adds (8320-elem L1)
                def fold_half(h, tag):
                    t6 = fold.tile([P, 6, ROW], F16, tag="t6")
                    nc.vector.tensor_add(t6[:], h[:, 0:6, :], h[:, 6:12, :])
                    t3 = fold.tile([P, 3, ROW], F16, tag="t3")
                    nc.vector.tensor_add(t3[:], t6[:, 0:3, :], t6[:, 3:6, :])
                    tt = small.tile([P, ROW], F16, tag=tag)
                    nc.vector.tensor_add(tt[:], t3[:, 0, :], t3[:, 1, :])
                    nc.vector.tensor_add(tt[:], tt[:], t3[:, 2, :])
                    return tt           # leftover col h[:,12,:] rides via PE

                ttA = fold_half(hA, "ttA")
                # PE accumulates halfA's fold, both leftover cols, into psum;
                # halfB's fold is added by DVE with the single PSUM read so
                # PE stays out of the tail's fold chain.
                pe_cols = [ttA[:], hA[:, 12, :], hB[:, 12, :]]
                for ci, col in enumerate(pe_cols):
                    stop = ci == len(pe_cols) - 1
                    nc.tensor.matmul(out=ps[:, 0:512], lhsT=ident[:],
                                     rhs=col[:, 0:512],
                                     start=False, stop=stop)
                    nc.tensor.matmul(out=ps[:, 512:ROW], lhsT=ident[:],
                                     rhs=col[:, 512:ROW],
                                     start=False, stop=stop)
                ttB = fold_half(hB, "ttB")

                # fld = psum chain + halfB col (single PSUM read on DVE)
                fld = fold.tile([P, ROW], F32, tag="fld")
                nc.vector.tensor_add(fld[:], ttB[:], ps[:])

                # --- FM identity epilogue for this chunk ---
                blk = fld[:, 0:VCOLS]             # [128, 624] = field_f
                sq = fold.tile([P, VCOLS], F32, tag="sq")
                q = small.tile([P, 1], F32, tag="q")
                nc.scalar.activation(
                    sq[:], blk, mybir.ActivationFunctionType.Square,
                    accum_out=q[:],
                )
                # s = sum over the 39 fields: strided reduce of [128,16,39]
                st = fold.tile([P, 16], F32, tag="st")
                blk_kf = blk.rearrange("p (f k) -> p k f", k=16)
                nc.vector.tensor_reduce(
                    out=st[:], in_=blk_kf, op=mybir.AluOpType.add,
                    axis=mybir.AxisListType.X,
                )
                s2 = small.tile([P, 16], F32, tag="s2")
                snorm = small.tile([P, 1], F32, tag="snorm")
                nc.scalar.activation(
                    s2[:], st[:], mybir.ActivationFunctionType.Square,
                    accum_out=snorm[:],
                )
                diff = small.tile([P, 1], F32, tag="diff")
                nc.vector.tensor_tensor(
                    out=diff[:], in0=snorm[:], in1=q[:],
                    op=mybir.AluOpType.subtract,
                )
                # first order = gathered w_hi sum + w_lo sum (+ dense part,
                # already in both cols via the dense matmul)
                wsum = small.tile([P, 1], F32, tag="wsum")
                nc.vector.tensor_add(
                    wsum[:], fld[:, VCOLS : VCOLS + 1],
                    fld[:, VCOLS + 1 : VCOLS + 2],
                )
                # out = 0.5*diff + first_order
                nc.scalar.activation(
                    res[:, c : c + 1],
                    diff[:],
                    mybir.ActivationFunctionType.Identity,
                    bias=wsum[:],
                    scale=0.5,
                )
                nc.sync.dma_start(out_t[:, c : c + 1], res[:, c : c + 1])

    nc.compile()
    return nc


def prep_inputs(dense_inputs, sparse_inputs, w0, w, v):
    """Host-side shard/pack: build per-core in_maps."""
    dense = np.asarray(dense_inputs, np.float32)
    sparse = np.asarray(sparse_inputs)
    w0 = np.asarray(w0, np.float32)
    w = np.asarray(w, np.float32)
    v = np.asarray(v, np.float32)

    vaug = np.zeros((FEATURE_NUM, ROW), np.float16)
    vaug[:, :VCOLS] = v.reshape(FEATURE_NUM, VCOLS).astype(np.float16)
    wfull = w[:, 0].copy()
    # fold w0 into sparse table 0 (each sample hits it exactly once)
    wfull[N_DENSE : N_DENSE + FEAT_PER_SPARSE] += w0[0]
    w_hi = wfull.astype(np.float16)
    w_lo = (wfull - w_hi.astype(np.float32)).astype(np.float16)
    vaug[:, VCOLS] = w_hi
    vaug[:, VCOLS + 1] = w_lo

    # dense rhs: [13, 640] fp32 -> hi/lo fp16 stacks
    vd = np.zeros((N_DENSE, ROW), np.float32)
    vd[:, :VCOLS] = v[:N_DENSE].reshape(N_DENSE, VCOLS)
    vd[:, VCOLS] = w[:N_DENSE, 0]
    vd_hi = vd.astype(np.float16)
    vd_lo = (vd - vd_hi.astype(np.float32)).astype(np.float16)
    vdense = np.concatenate([vd_hi, vd_hi, vd_lo], axis=0)  # [39, 640]

    offs = N_DENSE + FEAT_PER_SPARSE * np.arange(N_SPARSE, dtype=np.int64)
    gidx = (sparse.astype(np.int64) + offs[None, :]).astype(np.int16)  # [B, 26]

    in_maps = []
    for core in range(N_CORES):
        sl = slice(core * BC, (core + 1) * BC)
        x = dense[sl]                                    # [512, 13]
        x_hi = x.astype(np.float16)
        x_lo = (x - x_hi.astype(np.float32)).astype(np.float16)
        dt = np.ascontiguousarray(
            np.concatenate([x_hi.T, x_lo.T, x_hi.T], axis=0)
        )                                                # [39, 512]
        idxc = gidx[sl]                                  # [512, 26]
        buf = np.zeros((P, SCHUNKS * IDX_COLS_SC), np.int16)
        off_c = 0
        for c in range(SCHUNKS):
            rows = idxc[c * P : (c + 1) * P]            # [128, 26]
            fbase = 0
            for nf in groups_of(c):
                n = nf * P
                # call order: i = f_local*128 + p  ->  row idx[p, fbase+f]
                seg = np.ascontiguousarray(
                    rows[:, fbase : fbase + nf].T
                ).reshape(-1)                           # [nf*128]
                wrapped = seg.reshape(n // 16, 16).T    # [16, n/16]
                buf[:, off_c : off_c + n // 16] = np.tile(wrapped, (8, 1))
                fbase += nf
                off_c += n // 16
        in_maps.append({"vaug": vaug, "dense_t": dt, "vdense": vdense,
                        "idxs": buf,
                        "ident": np.eye(P, dtype=np.float16)})
    return in_maps


_NC_CACHE = None


def kernel(dense_inputs, sparse_inputs, w0, w, v):
    global _NC_CACHE
    if _NC_CACHE is None:
        _NC_CACHE = build_program()
    nc = _NC_CACHE
    in_maps = prep_inputs(dense_inputs, sparse_inputs, w0, w, v)
    res = run_bass_kernel_spmd(nc, in_maps, core_ids=list(range(N_CORES)))
    outs = []
    for r in res.results:
        o = r["out"]                                    # [128, 4]
        outs.append(np.ascontiguousarray(o.T).reshape(BC, 1))
    return np.concatenate(outs, axis=0).astype(np.float32)
